# revision 1
# baseline (speedup 1.0000x reference)
"""Distributed statevector Hadamard-gate kernel for 8 TRN2 NeuronCores.

Problem: y = U @ x where U = kron_{i=0..23}(M if i in (0,5,10,15,20) else I2),
x is a 2^24-amplitude complex64 statevector (qudit 0 = most significant axis),
M is the 2x2 Hadamard (real-valued).

Strategy
--------
M is real, so real/imag parts transform independently -> treat x as a float32
stream (interleaved re,im; bit-strides of qubit axes double).

Shard across 8 cores by qubits (1,2,3) (non-gate axes) -> every gate is local
to a core; no collectives. Per core: a 2^22-float slab whose bit layout is

  s = q0*2^21 q4*2^20 q5*2^19 q6*2^18 q7*2^17 q8*2^16 q9*2^15 q10*2^14
      q11*2^13 q12*2^12 q13*2^11 q14*2^10 q15*2^9 | q16..q23,reim (512-run)

On-chip layout: partition index p = q10*64 + q15*32 + q0*16 + q4*8 + q5*4
+ q6*2 + q7.  Gates on q0,q5,q10,q15 then become ONE 128x128 matmul with a
host-precomputed kron matrix L (entries +-s^5, the 5th gate's scale folded
in).  The q20 gate is a free-axis add/sub butterfly on the vector engine.
Single HBM pass: DMA-in -> DVE butterfly -> PE matmul (fp32) -> ACT copy
PSUM->SBUF -> DMA-out, pipelined over 16 x 1MB chunks per core.
"""

import math
import sys
import types

import numpy as np

import concourse.bass as bass
import concourse.mybir as mybir
from concourse.tile import TileContext
from concourse.bass_utils import run_bass_kernel_spmd


def _ensure_axon_hooks():
    """bass_utils' trace path does `from antenv.axon_hooks import ...`
    unconditionally; some images ship an `antenv` without that submodule,
    which would crash tracing.  Synthesize it (and register the ctypes NTFF
    hook when available) so tracing degrades gracefully instead.
    """
    try:
        import antenv.axon_hooks  # noqa: F401

        return
    except ImportError:
        pass
    try:
        import antenv
    except ImportError:
        return
    mod = types.ModuleType("antenv.axon_hooks")
    mod._hook = None

    def set_axon_ntff_profile_hook(hook):
        mod._hook = hook

    def get_axon_ntff_profile_hook():
        return mod._hook

    mod.set_axon_ntff_profile_hook = set_axon_ntff_profile_hook
    mod.get_axon_ntff_profile_hook = get_axon_ntff_profile_hook
    sys.modules["antenv.axon_hooks"] = mod
    antenv.axon_hooks = mod
    try:
        from trn_agent_boot.trn_boot import _ntff_profile_via_ctypes

        hook = _ntff_profile_via_ctypes("/opt/axon/libaxon_pjrt.so")
        if hook is not None:
            mod._hook = hook
    except Exception:
        pass


_ensure_axon_hooks()


def _legalize_waits(bir: dict) -> dict:
    """This image's walrus accepts only ONE sync-wait per TPB/DMA
    instruction; Tile emits up to ~4.  Hoist all but the last wait of each
    instruction into standalone EventSemaphore ops on the same engine,
    placed immediately before it — semantically identical (the engine
    blocks on them in program order).
    """
    for f in bir.get("functions", []):
        for b in f.get("blocks", []):
            out = []
            for i in b["instructions"]:
                si = i.get("sync_info") or {}
                waits = si.get("on_wait") or []
                if len(waits) > 1:
                    for k, wt in enumerate(waits[:-1]):
                        out.append({
                            "debug": i.get("debug", 0),
                            "engine": i["engine"],
                            "ins": [], "outs": [],
                            "name": f"hoistwait_{i['name']}_{k}",
                            "opcode": "EventSemaphore",
                            "sync_info": {"on_update": [], "on_wait": [wt]},
                        })
                    si["on_wait"] = [waits[-1]]
                out.append(i)
            b["instructions"] = out
    return bir


def _install_legalizer():
    import json as _json

    orig = bass.Bass.to_json_bytes
    if getattr(bass.Bass, "_wait_legalizer_installed", False):
        return

    def to_json_bytes(self, *a, **kw):
        raw = orig(self, *a, **kw)
        try:
            return _json.dumps(_legalize_waits(_json.loads(raw))).encode()
        except Exception:
            return raw

    bass.Bass.to_json_bytes = to_json_bytes
    bass.Bass._wait_legalizer_installed = True


_install_legalizer()

N_CORES = 8

_NC_CACHE: dict = {}

# set by kernel(): the BassKernelResults of the last run (exec_time_ns when
# run with BASS_TRACE=1) — used by the local test harness only
LAST_RESULT = None


def _build_nc(S: int, bfly):
    """Build the SPMD Bass program for one core.

    S: log2 of per-core slab float count (22 for complex64 input).
    bfly: ("had",) for add/sub butterfly (scale folded into L), or
          ("gen", a, b, c, d) for a generic real 2x2 q20 gate.
    """
    RUN = 1 << (S - 13)  # contiguous run (q16..q23[,reim]): 512 (cplx) / 256
    CHUNK_FREE = 2 * RUN  # per-partition free elems per chunk (q15, run)
    NCHUNKS = 32  # chunk bits: q9,q11,q12,q13,q14
    L_SUB = RUN // 32  # q21..q23[,reim] size below the q20 bit
    fp = mybir.dt.float32

    nc = bass.Bass()
    x = nc.declare_dram_parameter("x", [1 << S], fp, isOutput=False)
    w = nc.declare_dram_parameter("w", [128, 128], fp, isOutput=False)
    y = nc.declare_dram_parameter("y", [1 << S], fp, isOutput=True)

    # slab bits (MSB..LSB): P=(q0 q4 q5 q6 q7 q8), a=q9, t=q10, c=q11,
    # d=q12, e=q13, m=q14, f = (q15 run) contiguous 2*RUN.
    # Partition index p = P*2 + t  ->  DMA is 3-dim: (64, 2, 2*RUN).
    pat = "(P a t c d e m f) -> a c d e m P t f"
    dims = dict(P=64, a=2, t=2, c=2, d=2, e=2, m=2, f=CHUNK_FREE)
    xv = x[:].rearrange(pat, **dims)
    yv = y[:].rearrange(pat, **dims)

    with TileContext(nc) as tc:
        with (
            tc.tile_pool(name="wpool", bufs=1) as wpool,
            # one dedicated slot per chunk: in-DMAs never reuse a slot, so
            # they carry zero semaphore waits (walrus allows only one per
            # DMA pseudo-instruction)
            tc.tile_pool(name="inp", bufs=NCHUNKS) as inp,
            tc.tile_pool(name="bfp", bufs=3) as bfp,
            tc.tile_pool(name="b2p", bufs=3) as b2p,
            tc.tile_pool(name="outp", bufs=3) as outp,
            tc.tile_pool(name="psp", bufs=4, space="PSUM") as psp,
        ):
            wts = wpool.tile([128, 128], fp, tag="wstage")
            nc.sync.dma_start(out=wts[:], in_=w[:])
            # stage via DVE so matmuls' weight dep is on the DVE semaphore
            wt = wpool.tile([128, 128], fp, tag="wmain")
            nc.vector.tensor_copy(wt[:], wts[:])

            for g in range(NCHUNKS):
                ix = ((g >> 4) & 1, (g >> 3) & 1, (g >> 2) & 1, (g >> 1) & 1, g & 1)

                it = inp.tile([128, CHUNK_FREE], fp)
                nc.sync.dma_start(out=it[:], in_=xv[ix])

                # q15 butterfly: free = (q15, run) = (2, RUN)
                bf = bfp.tile([128, CHUNK_FREE], fp)
                iv = it[:].rearrange("p (w l) -> p w l", w=2, l=RUN)
                bv = bf[:].rearrange("p (w l) -> p w l", w=2, l=RUN)
                _bfly_pair(
                    nc, mybir, bfly,
                    bv[:, 0, :], bv[:, 1, :], iv[:, 0, :], iv[:, 1, :],
                )

                # q20 butterfly: free = (q15 q16..q19, q20, low) = (32, 2, L_SUB)
                b2 = b2p.tile([128, CHUNK_FREE], fp)
                jv = bf[:].rearrange("p (m w l) -> p m w l", m=32, w=2, l=L_SUB)
                ov = b2[:].rearrange("p (m w l) -> p m w l", m=32, w=2, l=L_SUB)
                _bfly_pair(
                    nc, mybir, bfly,
                    ov[:, :, 0, :], ov[:, :, 1, :], jv[:, :, 0, :], jv[:, :, 1, :],
                )

                ps = psp.tile([128, CHUNK_FREE], fp)
                ot = outp.tile([128, CHUNK_FREE], fp)
                for j in range(CHUNK_FREE // RUN):
                    sl = slice(j * RUN, (j + 1) * RUN)
                    nc.tensor.matmul(
                        ps[:, sl], wt[:], b2[:, sl], start=True, stop=True
                    )
                # PSUM evacuation on DVE: keeps every matmul dep on one sem
                nc.vector.tensor_copy(ot[:], ps[:])

                nc.sync.dma_start(out=yv[ix], in_=ot[:])
    return nc


def _bfly_pair(nc, mb, bfly, out0, out1, i0, i1):
    """Apply a 2x2 gate to the (i0, i1) pair of equally-shaped views."""
    if bfly[0] == "had":
        nc.vector.tensor_add(out0, i0, i1)
        nc.vector.tensor_sub(out1, i0, i1)
    else:
        _, ga, gb, gc, gd = bfly
        # out0 = ga*x0 + gb*x1 ; out1 = gc*x0 + gd*x1
        nc.vector.tensor_scalar_mul(out0, i0, float(ga))
        nc.vector.scalar_tensor_tensor(
            out0, i1, float(gb), out0, mb.AluOpType.mult, mb.AluOpType.add
        )
        nc.vector.tensor_scalar_mul(out1, i0, float(gc))
        nc.vector.scalar_tensor_tensor(
            out1, i1, float(gd), out1, mb.AluOpType.mult, mb.AluOpType.add
        )


def _get_nc(S: int, bfly):
    key = (S, bfly)
    if key not in _NC_CACHE:
        _NC_CACHE[key] = _build_nc(S, bfly)
    return _NC_CACHE[key]


def _build_L(Mr: np.ndarray, fold_scale: float) -> np.ndarray:
    """128x128 real matrix applying M on partition bits q0, q5, q10.

    Partition index p = q0*64 + q4*32 + q5*16 + q6*8 + q7*4 + q8*2 + q10.
    """
    I2 = np.eye(2, dtype=np.float64)
    L = np.array([[1.0]])
    for F in (Mr, I2, Mr, I2, I2, I2, Mr):  # q0, q4, q5, q6, q7, q8, q10
        L = np.kron(L, F)
    return (L * fold_scale).astype(np.float32)


def kernel(x: np.ndarray, M: np.ndarray) -> np.ndarray:
    x = np.asarray(x)
    M = np.asarray(M)
    n, batch = x.shape
    assert n == 1 << 24 and batch == 1, (n, batch)

    is_complex = np.iscomplexobj(x)
    if is_complex:
        xc = np.ascontiguousarray(x, dtype=np.complex64)
        xf = xc.reshape(-1).view(np.float32)
    else:
        xf = np.ascontiguousarray(x, dtype=np.float32).reshape(-1)
    F = xf.size
    S = int(round(math.log2(F))) - 3  # per-core slab = F/8 floats

    # gate matrix: must be (essentially) real
    Mc = np.asarray(M, dtype=np.complex128)
    assert np.abs(Mc.imag).max() <= 1e-5 * max(np.abs(Mc.real).max(), 1e-30), (
        "complex-valued M is not supported"
    )
    Mr = Mc.real.copy()

    s0 = Mr[0, 0]
    had_form = (
        abs(s0) > 0
        and abs(Mr[0, 1] - s0) <= 1e-6 * abs(s0)
        and abs(Mr[1, 0] - s0) <= 1e-6 * abs(s0)
        and abs(Mr[1, 1] + s0) <= 1e-6 * abs(s0)
    )
    if had_form:
        bfly = ("had",)
        L = _build_L(Mr, fold_scale=s0 * s0)  # two unnormalized butterflies
    else:
        bfly = ("gen", Mr[0, 0], Mr[0, 1], Mr[1, 0], Mr[1, 1])
        L = _build_L(Mr, fold_scale=1.0)
    wT = np.ascontiguousarray(L.T)  # lhsT[k, i] = L[i, k]

    nc = _get_nc(S, bfly if bfly[0] == "had" else bfly)

    # shard by qubits (1,2,3): xf.reshape(2[q0], 8[q1q2q3], F/16)
    xs = xf.reshape(2, 8, F // 16)
    in_maps = [
        {"x": np.ascontiguousarray(xs[:, cid, :]).reshape(-1), "w": wT}
        for cid in range(N_CORES)
    ]
    res = run_bass_kernel_spmd(nc, in_maps, list(range(N_CORES)))
    global LAST_RESULT
    LAST_RESULT = res
    outs = res.results

    yf = np.empty(F, dtype=np.float32)
    ys = yf.reshape(2, 8, F // 16)
    for cid in range(N_CORES):
        ys[:, cid, :] = outs[cid]["y"].reshape(2, F // 16)

    if is_complex:
        return yf.view(np.complex64).reshape(n, batch)
    return yf.reshape(n, batch)



# revision 4
# speedup vs baseline: 2.1792x; 2.1792x over previous
"""Distributed statevector Hadamard-gate kernel for 8 TRN2 NeuronCores.

Problem: y = U @ x where U = kron_{i=0..23}(M if i in (0,5,10,15,20) else I2),
x is a 2^24-amplitude complex64 statevector (qudit 0 = most significant axis),
M is the 2x2 Hadamard (real-valued).

Strategy
--------
M is real, so real/imag parts transform independently -> treat x as a float
stream (interleaved re,im; bit-strides of qubit axes double).

Shard across 8 cores by qubits (1,2,3) (non-gate axes) -> every gate is local
to a core; no collectives.  The rel-err budget is 2e-2, so the wire format is
fp16 (host casts fp32->fp16 on the way in and fp16->fp32 on the way out):
halves HBM traffic, doubles DVE throughput (2x_1P mode), and quadruples PE
throughput vs fp32.  Expected end-to-end rel err ~6e-4.

Per core: a 2^22-fp16 slab whose bit layout is (MSB..LSB)

  q0 q4 q5 q6 q7 q8 | q9 q10 q11 q12 | q13 q14 q15 | q16..q23,reim (512-run)

On-chip layout: partition index p = (q0 q4 q5 q6 q7 q8)*2 + q10.  Gates on
q0,q5,q10 become ONE 128x128 fp16 matmul with a host-precomputed kron matrix
L (entries +-s^3*s^2, the scale of the two unnormalized DVE butterflies
folded in).  q15 and q20 are free-axis add/sub butterflies on the vector
engine (fp16 -> 2x mode).  PSUM (fp32) is evacuated by the scalar engine
with a cast to fp16.  Single HBM pass: DMA-in -> DVE bfly x2 -> PE matmul ->
ACT copy/cast -> DMA-out, pipelined over 8 x 1MB chunks per core.
"""

import math
import sys
import types

import numpy as np

import concourse.bass as bass
import concourse.mybir as mybir
from concourse.tile import TileContext
from concourse.bass_utils import run_bass_kernel_spmd


def _ensure_axon_hooks():
    """bass_utils' trace path does `from antenv.axon_hooks import ...`
    unconditionally; some images ship an `antenv` without that submodule,
    which would crash tracing.  Synthesize it (and register the ctypes NTFF
    hook when available) so tracing degrades gracefully instead.
    """
    try:
        import antenv.axon_hooks  # noqa: F401

        return
    except ImportError:
        pass
    try:
        import antenv
    except ImportError:
        return
    mod = types.ModuleType("antenv.axon_hooks")
    mod._hook = None

    def set_axon_ntff_profile_hook(hook):
        mod._hook = hook

    def get_axon_ntff_profile_hook():
        return mod._hook

    mod.set_axon_ntff_profile_hook = set_axon_ntff_profile_hook
    mod.get_axon_ntff_profile_hook = get_axon_ntff_profile_hook
    sys.modules["antenv.axon_hooks"] = mod
    antenv.axon_hooks = mod
    try:
        from trn_agent_boot.trn_boot import _ntff_profile_via_ctypes

        hook = _ntff_profile_via_ctypes("/opt/axon/libaxon_pjrt.so")
        if hook is not None:
            mod._hook = hook
    except Exception:
        pass


_ensure_axon_hooks()


def _legalize_waits(bir: dict) -> dict:
    """This image's walrus accepts only ONE sync-wait per TPB/DMA
    instruction; Tile emits up to ~4.  Hoist all but the last wait of each
    instruction into standalone EventSemaphore ops on the same engine,
    placed immediately before it — semantically identical (the engine
    blocks on them in program order).
    """
    for f in bir.get("functions", []):
        for b in f.get("blocks", []):
            out = []
            for i in b["instructions"]:
                si = i.get("sync_info") or {}
                waits = si.get("on_wait") or []
                if len(waits) > 1:
                    for k, wt in enumerate(waits[:-1]):
                        out.append({
                            "debug": i.get("debug", 0),
                            "engine": i["engine"],
                            "ins": [], "outs": [],
                            "name": f"hoistwait_{i['name']}_{k}",
                            "opcode": "EventSemaphore",
                            "sync_info": {"on_update": [], "on_wait": [wt]},
                        })
                    si["on_wait"] = [waits[-1]]
                out.append(i)
            b["instructions"] = out
    return bir


def _install_legalizer():
    import json as _json

    orig = bass.Bass.to_json_bytes
    if getattr(bass.Bass, "_wait_legalizer_installed", False):
        return

    def to_json_bytes(self, *a, **kw):
        raw = orig(self, *a, **kw)
        try:
            return _json.dumps(_legalize_waits(_json.loads(raw))).encode()
        except Exception:
            return raw

    bass.Bass.to_json_bytes = to_json_bytes
    bass.Bass._wait_legalizer_installed = True


_install_legalizer()

N_CORES = 8

_NC_CACHE: dict = {}

# set by kernel(): the BassKernelResults of the last run (exec_time_ns when
# run with BASS_TRACE=1) — used by the local test harness only
LAST_RESULT = None


def _build_nc(S: int, bfly):
    """Build the SPMD Bass program for one core.

    S: log2 of per-core slab fp16-element count (22 for complex64 input).
    bfly: ("had",) for add/sub butterfly (scale folded into L), or
          ("gen", a, b, c, d) for a generic real 2x2 gate on q15/q20.
    """
    RUN = 1 << (S - 13)       # contiguous run (q16..q23[,reim]): 512 / 256
    CHUNK_FREE = 8 * RUN      # per-partition free elems (q13 q14 q15, run)
    NCHUNKS = 8               # chunk bits: q9, q11, q12
    L_SUB = RUN // 32         # q21..q23[,reim] size below the q20 bit
    fp16 = mybir.dt.float16
    fp32 = mybir.dt.float32

    nc = bass.Bass()
    x = nc.declare_dram_parameter("x", [1 << S], fp16, isOutput=False)
    w = nc.declare_dram_parameter("w", [128, 128], fp16, isOutput=False)
    y = nc.declare_dram_parameter("y", [1 << S], fp16, isOutput=True)

    # slab bits (MSB..LSB): P=(q0 q4 q5 q6 q7 q8), a=q9, t=q10, c=q11,
    # d=q12, f = (q13 q14 q15 run) contiguous 8*RUN.
    # Partition index p = P*2 + t  ->  DMA is 3-dim: (64, 2, 8*RUN).
    pat = "(P a t c d f) -> a c d P t f"
    dims = dict(P=64, a=2, t=2, c=2, d=2, f=CHUNK_FREE)
    xv = x[:].rearrange(pat, **dims)
    yv = y[:].rearrange(pat, **dims)

    with TileContext(nc) as tc:
        with (
            tc.tile_pool(name="wpool", bufs=1) as wpool,
            # one dedicated slot per chunk: in-DMAs never reuse a slot, so
            # they carry zero semaphore waits (walrus allows only one per
            # DMA pseudo-instruction)
            tc.tile_pool(name="inp", bufs=NCHUNKS) as inp,
            tc.tile_pool(name="bfp", bufs=2) as bfp,
            tc.tile_pool(name="b2p", bufs=2) as b2p,
            tc.tile_pool(name="outp", bufs=3) as outp,
            tc.tile_pool(name="psp", bufs=4, space="PSUM") as psp,
        ):
            wts = wpool.tile([128, 128], fp16, tag="wstage")
            nc.sync.dma_start(out=wts[:], in_=w[:])
            # stage via DVE so matmuls' weight dep is on the DVE semaphore
            wt = wpool.tile([128, 128], fp16, tag="wmain")
            nc.vector.tensor_copy(wt[:], wts[:])

            for g in range(NCHUNKS):
                ix = ((g >> 2) & 1, (g >> 1) & 1, g & 1)

                it = inp.tile([128, CHUNK_FREE], fp16)
                nc.sync.dma_start(out=it[:], in_=xv[ix])

                # q15 butterfly: free = (q13 q14, q15, run) = (4, 2, RUN)
                bf = bfp.tile([128, CHUNK_FREE], fp16)
                iv = it[:].rearrange("p (m w l) -> p m w l", m=4, w=2, l=RUN)
                bv = bf[:].rearrange("p (m w l) -> p m w l", m=4, w=2, l=RUN)
                _bfly_pair(
                    nc, mybir, bfly,
                    bv[:, :, 0, :], bv[:, :, 1, :], iv[:, :, 0, :], iv[:, :, 1, :],
                )

                # q20 butterfly: free = (q13..q19, q20, low) = (m2, 2, L_SUB)
                m2 = CHUNK_FREE // (2 * L_SUB)
                b2 = b2p.tile([128, CHUNK_FREE], fp16)
                jv = bf[:].rearrange("p (m w l) -> p m w l", m=m2, w=2, l=L_SUB)
                ov = b2[:].rearrange("p (m w l) -> p m w l", m=m2, w=2, l=L_SUB)
                _bfly_pair(
                    nc, mybir, bfly,
                    ov[:, :, 0, :], ov[:, :, 1, :], jv[:, :, 0, :], jv[:, :, 1, :],
                )

                ot = outp.tile([128, CHUNK_FREE], fp16)
                for j in range(CHUNK_FREE // 1024):
                    sl = slice(j * 1024, (j + 1) * 1024)
                    ps = psp.tile([128, 1024], fp32)
                    # moving-operand ISA cap is 512 elems -> 2 matmuls per
                    # 2-bank PSUM tile
                    for h in range(2):
                        hs = slice(h * 512, (h + 1) * 512)
                        nc.tensor.matmul(
                            ps[:, hs], wt[:],
                            b2[:, j * 1024 + h * 512:j * 1024 + (h + 1) * 512],
                            start=True, stop=True,
                        )
                    # PSUM evacuation + fp32->fp16 cast on the (otherwise
                    # idle) scalar engine
                    nc.scalar.copy(ot[:, sl], ps[:])

                nc.sync.dma_start(out=yv[ix], in_=ot[:])
    return nc


def _bfly_pair(nc, mb, bfly, out0, out1, i0, i1):
    """Apply a 2x2 gate to the (i0, i1) pair of equally-shaped views."""
    if bfly[0] == "had":
        nc.vector.tensor_add(out0, i0, i1)
        nc.vector.tensor_sub(out1, i0, i1)
    else:
        _, ga, gb, gc, gd = bfly
        # out0 = ga*x0 + gb*x1 ; out1 = gc*x0 + gd*x1
        nc.vector.tensor_scalar_mul(out0, i0, float(ga))
        nc.vector.scalar_tensor_tensor(
            out0, i1, float(gb), out0, mb.AluOpType.mult, mb.AluOpType.add
        )
        nc.vector.tensor_scalar_mul(out1, i0, float(gc))
        nc.vector.scalar_tensor_tensor(
            out1, i1, float(gd), out1, mb.AluOpType.mult, mb.AluOpType.add
        )


def _get_nc(S: int, bfly):
    key = (S, bfly)
    if key not in _NC_CACHE:
        _NC_CACHE[key] = _build_nc(S, bfly)
    return _NC_CACHE[key]


def _build_L(Mr: np.ndarray, fold_scale: float) -> np.ndarray:
    """128x128 real matrix applying M on partition bits q0, q5, q10.

    Partition index p = q0*64 + q4*32 + q5*16 + q6*8 + q7*4 + q8*2 + q10.
    """
    I2 = np.eye(2, dtype=np.float64)
    L = np.array([[1.0]])
    for F in (Mr, I2, Mr, I2, I2, I2, Mr):  # q0, q4, q5, q6, q7, q8, q10
        L = np.kron(L, F)
    return L * fold_scale


def kernel(x: np.ndarray, M: np.ndarray) -> np.ndarray:
    x = np.asarray(x)
    M = np.asarray(M)
    n, batch = x.shape
    assert n == 1 << 24 and batch == 1, (n, batch)

    is_complex = np.iscomplexobj(x)
    if is_complex:
        xc = np.ascontiguousarray(x, dtype=np.complex64)
        xf = xc.reshape(-1).view(np.float32)
    else:
        xf = np.ascontiguousarray(x, dtype=np.float32).reshape(-1)
    xh = xf.astype(np.float16)  # wire format: fp16 (tolerance is 2e-2)
    F = xh.size
    S = int(round(math.log2(F))) - 3  # per-core slab = F/8 elems

    # gate matrix: must be (essentially) real
    Mc = np.asarray(M, dtype=np.complex128)
    assert np.abs(Mc.imag).max() <= 1e-5 * max(np.abs(Mc.real).max(), 1e-30), (
        "complex-valued M is not supported"
    )
    Mr = Mc.real.copy()

    s0 = Mr[0, 0]
    had_form = (
        abs(s0) > 0
        and abs(Mr[0, 1] - s0) <= 1e-6 * abs(s0)
        and abs(Mr[1, 0] - s0) <= 1e-6 * abs(s0)
        and abs(Mr[1, 1] + s0) <= 1e-6 * abs(s0)
    )
    if had_form:
        bfly = ("had",)
        L = _build_L(Mr, fold_scale=s0 * s0)  # two unnormalized butterflies
    else:
        bfly = ("gen", Mr[0, 0], Mr[0, 1], Mr[1, 0], Mr[1, 1])
        L = _build_L(Mr, fold_scale=1.0)
    wT = np.ascontiguousarray(L.T.astype(np.float16))  # lhsT[k, i] = L[i, k]

    nc = _get_nc(S, bfly if bfly[0] == "had" else bfly)

    # shard by qubits (1,2,3): xh.reshape(2[q0], 8[q1q2q3], F/16)
    xs = xh.reshape(2, 8, F // 16)
    in_maps = [
        {"x": np.ascontiguousarray(xs[:, cid, :]).reshape(-1), "w": wT}
        for cid in range(N_CORES)
    ]
    res = run_bass_kernel_spmd(nc, in_maps, list(range(N_CORES)))
    global LAST_RESULT
    LAST_RESULT = res
    outs = res.results

    yf = np.empty(F, dtype=np.float32)
    ys = yf.reshape(2, 8, F // 16)
    for cid in range(N_CORES):
        ys[:, cid, :] = outs[cid]["y"].reshape(2, F // 16).astype(np.float32)

    if is_complex:
        return yf.view(np.complex64).reshape(n, batch)
    return yf.reshape(n, batch)


# revision 5
# speedup vs baseline: 2.4841x; 1.1399x over previous
"""Distributed statevector Hadamard-gate kernel for 8 TRN2 NeuronCores.

Problem: y = U @ x where U = kron_{i=0..23}(M if i in (0,5,10,15,20) else I2),
x is a 2^24-amplitude complex64 statevector (qudit 0 = most significant axis),
M is the 2x2 Hadamard (real-valued).

Strategy
--------
M is real, so real/imag parts transform independently -> treat x as a float
stream (interleaved re,im; bit-strides of qubit axes double).

Shard across 8 cores by qubits (1,2,3) (non-gate axes) -> every gate is local
to a core; no collectives.  The rel-err budget is 2e-2, so the wire format is
fp16 (host casts fp32->fp16 on the way in and fp16->fp32 on the way out):
halves HBM traffic, doubles DVE throughput (2x_1P mode), and quadruples PE
throughput vs fp32.  Expected end-to-end rel err ~6e-4.

Per core: a 2^22-fp16 slab whose bit layout is (MSB..LSB)

  q0 q4 q5 q6 q7 q8 | q9 q10 q11 q12 | q13 q14 q15 | q16..q23,reim (512-run)

On-chip layout: partition index p = (q0 q4 q5 q6 q7 q8)*2 + q10.  Gates on
q0,q5,q10 become ONE 128x128 fp16 matmul with a host-precomputed kron matrix
L (entries +-s^3*s^2, the scale of the two unnormalized DVE butterflies
folded in).  q15 and q20 are free-axis add/sub butterflies on the vector
engine (fp16 -> 2x mode).  PSUM (fp32) is evacuated by the scalar engine
with a cast to fp16.  Single HBM pass: DMA-in -> DVE bfly x2 -> PE matmul ->
ACT copy/cast -> DMA-out, pipelined over 8 x 1MB chunks per core.
"""

import math
import sys
import types

import numpy as np

import concourse.bass as bass
import concourse.mybir as mybir
from concourse.tile import TileContext
from concourse.bass_utils import run_bass_kernel_spmd


def _ensure_axon_hooks():
    """bass_utils' trace path does `from antenv.axon_hooks import ...`
    unconditionally; some images ship an `antenv` without that submodule,
    which would crash tracing.  Synthesize it (and register the ctypes NTFF
    hook when available) so tracing degrades gracefully instead.
    """
    try:
        import antenv.axon_hooks  # noqa: F401

        return
    except ImportError:
        pass
    try:
        import antenv
    except ImportError:
        return
    mod = types.ModuleType("antenv.axon_hooks")
    mod._hook = None

    def set_axon_ntff_profile_hook(hook):
        mod._hook = hook

    def get_axon_ntff_profile_hook():
        return mod._hook

    mod.set_axon_ntff_profile_hook = set_axon_ntff_profile_hook
    mod.get_axon_ntff_profile_hook = get_axon_ntff_profile_hook
    sys.modules["antenv.axon_hooks"] = mod
    antenv.axon_hooks = mod
    try:
        from trn_agent_boot.trn_boot import _ntff_profile_via_ctypes

        hook = _ntff_profile_via_ctypes("/opt/axon/libaxon_pjrt.so")
        if hook is not None:
            mod._hook = hook
    except Exception:
        pass


_ensure_axon_hooks()


def _legalize_waits(bir: dict) -> dict:
    """This image's walrus accepts only ONE sync-wait per TPB/DMA
    instruction; Tile emits up to ~4.  Hoist all but the last wait of each
    instruction into standalone EventSemaphore ops on the same engine,
    placed immediately before it — semantically identical (the engine
    blocks on them in program order).
    """
    for f in bir.get("functions", []):
        for b in f.get("blocks", []):
            out = []
            for i in b["instructions"]:
                si = i.get("sync_info") or {}
                waits = si.get("on_wait") or []
                if len(waits) > 1:
                    for k, wt in enumerate(waits[:-1]):
                        out.append({
                            "debug": i.get("debug", 0),
                            "engine": i["engine"],
                            "ins": [], "outs": [],
                            "name": f"hoistwait_{i['name']}_{k}",
                            "opcode": "EventSemaphore",
                            "sync_info": {"on_update": [], "on_wait": [wt]},
                        })
                    si["on_wait"] = [waits[-1]]
                out.append(i)
            b["instructions"] = out
    return bir


def _install_legalizer():
    import json as _json

    orig = bass.Bass.to_json_bytes
    if getattr(bass.Bass, "_wait_legalizer_installed", False):
        return

    def to_json_bytes(self, *a, **kw):
        raw = orig(self, *a, **kw)
        try:
            return _json.dumps(_legalize_waits(_json.loads(raw))).encode()
        except Exception:
            return raw

    bass.Bass.to_json_bytes = to_json_bytes
    bass.Bass._wait_legalizer_installed = True


_install_legalizer()

N_CORES = 8

_NC_CACHE: dict = {}

# set by kernel(): the BassKernelResults of the last run (exec_time_ns when
# run with BASS_TRACE=1) — used by the local test harness only
LAST_RESULT = None


def _build_nc(S: int, bfly):
    """Build the SPMD Bass program for one core.

    S: log2 of per-core slab fp16-element count (22 for complex64 input).
    bfly: ("had",) for add/sub butterfly (scale folded into L), or
          ("gen", a, b, c, d) for a generic real 2x2 gate on q15/q20.
    """
    RUN = 1 << (S - 13)       # contiguous run (q16..q23[,reim]): 512 / 256
    CHUNK_FREE = 8 * RUN      # per-partition free elems (q13 q14 q15, run)
    NCHUNKS = 8               # chunk bits: q9, q11, q12
    L_SUB = RUN // 32         # q21..q23[,reim] size below the q20 bit
    fp16 = mybir.dt.float16
    fp32 = mybir.dt.float32

    nc = bass.Bass()
    x = nc.declare_dram_parameter("x", [1 << S], fp16, isOutput=False)
    w = nc.declare_dram_parameter("w", [128, 128], fp16, isOutput=False)
    y = nc.declare_dram_parameter("y", [1 << S], fp16, isOutput=True)

    # slab bits (MSB..LSB): P=(q0 q4 q5 q6 q7 q8), a=q9, t=q10, c=q11,
    # d=q12, f = (q13 q14 q15 run) contiguous 8*RUN.
    # Partition index p = P*2 + t  ->  DMA is 3-dim: (64, 2, 8*RUN).
    pat = "(P a t c d f) -> a c d P t f"
    dims = dict(P=64, a=2, t=2, c=2, d=2, f=CHUNK_FREE)
    xv = x[:].rearrange(pat, **dims)
    yv = y[:].rearrange(pat, **dims)

    with TileContext(nc) as tc:
        with (
            tc.tile_pool(name="wpool", bufs=1) as wpool,
            # one dedicated slot per chunk: in-DMAs never reuse a slot, so
            # they carry zero semaphore waits (walrus allows only one per
            # DMA pseudo-instruction)
            tc.tile_pool(name="inp", bufs=NCHUNKS) as inp,
            tc.tile_pool(name="bfp", bufs=2) as bfp,
            tc.tile_pool(name="b2p", bufs=2) as b2p,
            tc.tile_pool(name="outp", bufs=3) as outp,
            tc.tile_pool(name="psp", bufs=4, space="PSUM") as psp,
        ):
            wts = wpool.tile([128, 128], fp16, tag="wstage")
            nc.sync.dma_start(out=wts[:], in_=w[:])
            # stage via DVE so matmuls' weight dep is on the DVE semaphore
            wt = wpool.tile([128, 128], fp16, tag="wmain")
            nc.vector.tensor_copy(wt[:], wts[:])

            for g in range(NCHUNKS):
                ix = ((g >> 2) & 1, (g >> 1) & 1, g & 1)

                it = inp.tile([128, CHUNK_FREE], fp16)
                nc.sync.dma_start(out=it[:], in_=xv[ix])

                # q15 butterfly: free = (q13 q14, q15, run) = (4, 2, RUN)
                bf = bfp.tile([128, CHUNK_FREE], fp16)
                iv = it[:].rearrange("p (m w l) -> p m w l", m=4, w=2, l=RUN)
                bv = bf[:].rearrange("p (m w l) -> p m w l", m=4, w=2, l=RUN)
                _bfly_pair(
                    nc, mybir, bfly,
                    bv[:, :, 0, :], bv[:, :, 1, :], iv[:, :, 0, :], iv[:, :, 1, :],
                )

                # q20 butterfly: free = (q13..q19, q20, low) = (m2, 2, L_SUB)
                m2 = CHUNK_FREE // (2 * L_SUB)
                b2 = b2p.tile([128, CHUNK_FREE], fp16)
                jv = bf[:].rearrange("p (m w l) -> p m w l", m=m2, w=2, l=L_SUB)
                ov = b2[:].rearrange("p (m w l) -> p m w l", m=m2, w=2, l=L_SUB)
                _bfly_pair(
                    nc, mybir, bfly,
                    ov[:, :, 0, :], ov[:, :, 1, :], jv[:, :, 0, :], jv[:, :, 1, :],
                )

                ot = outp.tile([128, CHUNK_FREE], fp16)
                for j in range(CHUNK_FREE // 1024):
                    sl = slice(j * 1024, (j + 1) * 1024)
                    ps = psp.tile([128, 1024], fp32)
                    # moving-operand ISA cap is 512 elems -> 2 matmuls per
                    # 2-bank PSUM tile
                    for h in range(2):
                        hs = slice(h * 512, (h + 1) * 512)
                        nc.tensor.matmul(
                            ps[:, hs], wt[:],
                            b2[:, j * 1024 + h * 512:j * 1024 + (h + 1) * 512],
                            start=True, stop=True,
                        )
                    # PSUM evacuation + fp32->fp16 cast on the (otherwise
                    # idle) scalar engine
                    nc.scalar.copy(ot[:, sl], ps[:])

                # out-DMAs go on the ACT HWDGE ring: they wait on compute,
                # and in the SP FIFO they would stall later in-DMAs
                nc.scalar.dma_start(out=yv[ix], in_=ot[:])
    return nc


def _bfly_pair(nc, mb, bfly, out0, out1, i0, i1):
    """Apply a 2x2 gate to the (i0, i1) pair of equally-shaped views."""
    if bfly[0] == "had":
        nc.vector.tensor_add(out0, i0, i1)
        nc.vector.tensor_sub(out1, i0, i1)
    else:
        _, ga, gb, gc, gd = bfly
        # out0 = ga*x0 + gb*x1 ; out1 = gc*x0 + gd*x1
        nc.vector.tensor_scalar_mul(out0, i0, float(ga))
        nc.vector.scalar_tensor_tensor(
            out0, i1, float(gb), out0, mb.AluOpType.mult, mb.AluOpType.add
        )
        nc.vector.tensor_scalar_mul(out1, i0, float(gc))
        nc.vector.scalar_tensor_tensor(
            out1, i1, float(gd), out1, mb.AluOpType.mult, mb.AluOpType.add
        )


def _get_nc(S: int, bfly):
    key = (S, bfly)
    if key not in _NC_CACHE:
        _NC_CACHE[key] = _build_nc(S, bfly)
    return _NC_CACHE[key]


def _build_L(Mr: np.ndarray, fold_scale: float) -> np.ndarray:
    """128x128 real matrix applying M on partition bits q0, q5, q10.

    Partition index p = q0*64 + q4*32 + q5*16 + q6*8 + q7*4 + q8*2 + q10.
    """
    I2 = np.eye(2, dtype=np.float64)
    L = np.array([[1.0]])
    for F in (Mr, I2, Mr, I2, I2, I2, Mr):  # q0, q4, q5, q6, q7, q8, q10
        L = np.kron(L, F)
    return L * fold_scale


def kernel(x: np.ndarray, M: np.ndarray) -> np.ndarray:
    x = np.asarray(x)
    M = np.asarray(M)
    n, batch = x.shape
    assert n == 1 << 24 and batch == 1, (n, batch)

    is_complex = np.iscomplexobj(x)
    if is_complex:
        xc = np.ascontiguousarray(x, dtype=np.complex64)
        xf = xc.reshape(-1).view(np.float32)
    else:
        xf = np.ascontiguousarray(x, dtype=np.float32).reshape(-1)
    xh = xf.astype(np.float16)  # wire format: fp16 (tolerance is 2e-2)
    F = xh.size
    S = int(round(math.log2(F))) - 3  # per-core slab = F/8 elems

    # gate matrix: must be (essentially) real
    Mc = np.asarray(M, dtype=np.complex128)
    assert np.abs(Mc.imag).max() <= 1e-5 * max(np.abs(Mc.real).max(), 1e-30), (
        "complex-valued M is not supported"
    )
    Mr = Mc.real.copy()

    s0 = Mr[0, 0]
    had_form = (
        abs(s0) > 0
        and abs(Mr[0, 1] - s0) <= 1e-6 * abs(s0)
        and abs(Mr[1, 0] - s0) <= 1e-6 * abs(s0)
        and abs(Mr[1, 1] + s0) <= 1e-6 * abs(s0)
    )
    if had_form:
        bfly = ("had",)
        L = _build_L(Mr, fold_scale=s0 * s0)  # two unnormalized butterflies
    else:
        bfly = ("gen", Mr[0, 0], Mr[0, 1], Mr[1, 0], Mr[1, 1])
        L = _build_L(Mr, fold_scale=1.0)
    wT = np.ascontiguousarray(L.T.astype(np.float16))  # lhsT[k, i] = L[i, k]

    nc = _get_nc(S, bfly if bfly[0] == "had" else bfly)

    # shard by qubits (1,2,3): xh.reshape(2[q0], 8[q1q2q3], F/16)
    xs = xh.reshape(2, 8, F // 16)
    in_maps = [
        {"x": np.ascontiguousarray(xs[:, cid, :]).reshape(-1), "w": wT}
        for cid in range(N_CORES)
    ]
    res = run_bass_kernel_spmd(nc, in_maps, list(range(N_CORES)))
    global LAST_RESULT
    LAST_RESULT = res
    outs = res.results

    yf = np.empty(F, dtype=np.float32)
    ys = yf.reshape(2, 8, F // 16)
    for cid in range(N_CORES):
        ys[:, cid, :] = outs[cid]["y"].reshape(2, F // 16).astype(np.float32)

    if is_complex:
        return yf.view(np.complex64).reshape(n, batch)
    return yf.reshape(n, batch)


# revision 6
# speedup vs baseline: 2.5345x; 1.0203x over previous
"""Distributed statevector Hadamard-gate kernel for 8 TRN2 NeuronCores.

Problem: y = U @ x where U = kron_{i=0..23}(M if i in (0,5,10,15,20) else I2),
x is a 2^24-amplitude complex64 statevector (qudit 0 = most significant axis),
M is the 2x2 Hadamard (real-valued).

Strategy
--------
M is real, so real/imag parts transform independently -> treat x as a float
stream (interleaved re,im; bit-strides of qubit axes double).

Shard across 8 cores by qubits (1,2,3) (non-gate axes) -> every gate is local
to a core; no collectives.  The rel-err budget is 2e-2, so the wire format is
fp16 (host casts fp32->fp16 on the way in and fp16->fp32 on the way out):
halves HBM traffic, doubles DVE throughput (2x_1P mode), and quadruples PE
throughput vs fp32.  Expected end-to-end rel err ~6e-4.

Per core: a 2^22-fp16 slab whose bit layout is (MSB..LSB)

  q0 q4 q5 q6 q7 q8 | q9 q10 q11 q12 | q13 q14 q15 | q16..q23,reim (512-run)

On-chip layout: partition index p = (q0 q4 q5 q6 q7 q8)*2 + q10.  Gates on
q0,q5,q10 become ONE 128x128 fp16 matmul with a host-precomputed kron matrix
L (entries +-s^3*s^2, the scale of the two unnormalized DVE butterflies
folded in).  q15 and q20 are free-axis add/sub butterflies on the vector
engine (fp16 -> 2x mode).  PSUM (fp32) is evacuated by the scalar engine
with a cast to fp16.  Single HBM pass: DMA-in -> DVE bfly x2 -> PE matmul ->
ACT copy/cast -> DMA-out, pipelined over 8 x 1MB chunks per core.
"""

import math
import sys
import types

import numpy as np

import concourse.bass as bass
import concourse.mybir as mybir
from concourse.tile import TileContext
from concourse.bass_utils import run_bass_kernel_spmd


def _ensure_axon_hooks():
    """bass_utils' trace path does `from antenv.axon_hooks import ...`
    unconditionally; some images ship an `antenv` without that submodule,
    which would crash tracing.  Synthesize it (and register the ctypes NTFF
    hook when available) so tracing degrades gracefully instead.
    """
    try:
        import antenv.axon_hooks  # noqa: F401

        return
    except ImportError:
        pass
    try:
        import antenv
    except ImportError:
        return
    mod = types.ModuleType("antenv.axon_hooks")
    mod._hook = None

    def set_axon_ntff_profile_hook(hook):
        mod._hook = hook

    def get_axon_ntff_profile_hook():
        return mod._hook

    mod.set_axon_ntff_profile_hook = set_axon_ntff_profile_hook
    mod.get_axon_ntff_profile_hook = get_axon_ntff_profile_hook
    sys.modules["antenv.axon_hooks"] = mod
    antenv.axon_hooks = mod
    try:
        from trn_agent_boot.trn_boot import _ntff_profile_via_ctypes

        hook = _ntff_profile_via_ctypes("/opt/axon/libaxon_pjrt.so")
        if hook is not None:
            mod._hook = hook
    except Exception:
        pass


_ensure_axon_hooks()


def _legalize_waits(bir: dict) -> dict:
    """This image's walrus accepts only ONE sync-wait per TPB/DMA
    instruction; Tile emits up to ~4.  Hoist all but the last wait of each
    instruction into standalone EventSemaphore ops on the same engine,
    placed immediately before it — semantically identical (the engine
    blocks on them in program order).
    """
    for f in bir.get("functions", []):
        for b in f.get("blocks", []):
            out = []
            for i in b["instructions"]:
                si = i.get("sync_info") or {}
                waits = si.get("on_wait") or []
                if len(waits) > 1:
                    for k, wt in enumerate(waits[:-1]):
                        out.append({
                            "debug": i.get("debug", 0),
                            "engine": i["engine"],
                            "ins": [], "outs": [],
                            "name": f"hoistwait_{i['name']}_{k}",
                            "opcode": "EventSemaphore",
                            "sync_info": {"on_update": [], "on_wait": [wt]},
                        })
                    si["on_wait"] = [waits[-1]]
                out.append(i)
            b["instructions"] = out
    return bir


def _install_legalizer():
    import json as _json

    orig = bass.Bass.to_json_bytes
    if getattr(bass.Bass, "_wait_legalizer_installed", False):
        return

    def to_json_bytes(self, *a, **kw):
        raw = orig(self, *a, **kw)
        try:
            return _json.dumps(_legalize_waits(_json.loads(raw))).encode()
        except Exception:
            return raw

    bass.Bass.to_json_bytes = to_json_bytes
    bass.Bass._wait_legalizer_installed = True


_install_legalizer()

N_CORES = 8

_NC_CACHE: dict = {}

# set by kernel(): the BassKernelResults of the last run (exec_time_ns when
# run with BASS_TRACE=1) — used by the local test harness only
LAST_RESULT = None


def _build_nc(S: int, bfly):
    """Build the SPMD Bass program for one core.

    S: log2 of per-core slab fp16-element count (22 for complex64 input).
    bfly: ("had",) for add/sub butterfly (scale folded into L), or
          ("gen", a, b, c, d) for a generic real 2x2 gate on q15/q20.
    """
    RUN = 1 << (S - 13)       # contiguous run (q16..q23[,reim]): 512 / 256
    IN_FREE = 4 * RUN         # per-partition free elems of an in-chunk
    OUT_FREE = 8 * RUN        # out chunks pair up two in-chunks
    NOUT = 8                  # out-chunk bits: q9, q11, q12
    L_SUB = RUN // 32         # q21..q23[,reim] size below the q20 bit
    fp16 = mybir.dt.float16
    fp32 = mybir.dt.float32

    nc = bass.Bass()
    x = nc.declare_dram_parameter("x", [1 << S], fp16, isOutput=False)
    w = nc.declare_dram_parameter("w", [128, 128], fp16, isOutput=False)
    y = nc.declare_dram_parameter("y", [1 << S], fp16, isOutput=True)

    # slab bits (MSB..LSB): P=(q0 q4 q5 q6 q7 q8), a=q9, t=q10, c=q11,
    # d=q12, e=q13, f = (q14 q15 run) contiguous 4*RUN.
    # Partition index p = P*2 + t.
    xv = x[:].rearrange(
        "(P a t c d e f) -> a c d e P t f",
        P=64, a=2, t=2, c=2, d=2, e=2, f=IN_FREE,
    )
    yv = y[:].rearrange(
        "(P a t c d f) -> a c d P t f",
        P=64, a=2, t=2, c=2, d=2, f=OUT_FREE,
    )

    with TileContext(nc) as tc:
        with (
            tc.tile_pool(name="wpool", bufs=1) as wpool,
            # one dedicated slot per chunk: in-DMAs never reuse a slot, so
            # they carry zero semaphore waits (walrus allows only one per
            # DMA pseudo-instruction)
            tc.tile_pool(name="inp", bufs=2 * NOUT) as inp,
            tc.tile_pool(name="bfp", bufs=3) as bfp,
            tc.tile_pool(name="b2p", bufs=3) as b2p,
            tc.tile_pool(name="outp", bufs=3) as outp,
            tc.tile_pool(name="psp", bufs=4, space="PSUM") as psp,
        ):
            wts = wpool.tile([128, 128], fp16, tag="wstage")
            nc.sync.dma_start(out=wts[:], in_=w[:])
            # stage via DVE so matmuls' weight dep is on the DVE semaphore
            wt = wpool.tile([128, 128], fp16, tag="wmain")
            nc.vector.tensor_copy(wt[:], wts[:])

            for g in range(NOUT):
                gx = ((g >> 2) & 1, (g >> 1) & 1, g & 1)
                ot = outp.tile([128, OUT_FREE], fp16)

                for e in range(2):
                    it = inp.tile([128, IN_FREE], fp16)
                    nc.sync.dma_start(out=it[:], in_=xv[gx + (e,)])

                    # q15 butterfly: free = (q14, q15, run) = (2, 2, RUN)
                    bf = bfp.tile([128, IN_FREE], fp16)
                    iv = it[:].rearrange("p (m w l) -> p m w l", m=2, w=2, l=RUN)
                    bv = bf[:].rearrange("p (m w l) -> p m w l", m=2, w=2, l=RUN)
                    _bfly_pair(
                        nc, mybir, bfly,
                        bv[:, :, 0, :], bv[:, :, 1, :],
                        iv[:, :, 0, :], iv[:, :, 1, :],
                    )

                    # q20 butterfly: free = (q14..q19, q20, low)
                    m2 = IN_FREE // (2 * L_SUB)
                    b2 = b2p.tile([128, IN_FREE], fp16)
                    jv = bf[:].rearrange("p (m w l) -> p m w l", m=m2, w=2, l=L_SUB)
                    ov = b2[:].rearrange("p (m w l) -> p m w l", m=m2, w=2, l=L_SUB)
                    _bfly_pair(
                        nc, mybir, bfly,
                        ov[:, :, 0, :], ov[:, :, 1, :],
                        jv[:, :, 0, :], jv[:, :, 1, :],
                    )

                    for j in range(IN_FREE // 1024):
                        ps = psp.tile([128, 1024], fp32)
                        # moving-operand ISA cap is 512 elems -> 2 matmuls
                        # per 2-bank PSUM tile
                        for h in range(2):
                            nc.tensor.matmul(
                                ps[:, h * 512:(h + 1) * 512], wt[:],
                                b2[:, j * 1024 + h * 512:j * 1024 + (h + 1) * 512],
                                start=True, stop=True,
                            )
                        # PSUM evacuation + fp32->fp16 cast on the
                        # (otherwise idle) scalar engine
                        nc.scalar.copy(
                            ot[:, e * IN_FREE + j * 1024:
                               e * IN_FREE + (j + 1) * 1024],
                            ps[:],
                        )

                # out-DMAs go on the ACT HWDGE ring: they wait on compute,
                # and in the SP FIFO they would stall later in-DMAs
                nc.scalar.dma_start(out=yv[gx], in_=ot[:])
    return nc


def _bfly_pair(nc, mb, bfly, out0, out1, i0, i1):
    """Apply a 2x2 gate to the (i0, i1) pair of equally-shaped views."""
    if bfly[0] == "had":
        nc.vector.tensor_add(out0, i0, i1)
        nc.vector.tensor_sub(out1, i0, i1)
    else:
        _, ga, gb, gc, gd = bfly
        # out0 = ga*x0 + gb*x1 ; out1 = gc*x0 + gd*x1
        nc.vector.tensor_scalar_mul(out0, i0, float(ga))
        nc.vector.scalar_tensor_tensor(
            out0, i1, float(gb), out0, mb.AluOpType.mult, mb.AluOpType.add
        )
        nc.vector.tensor_scalar_mul(out1, i0, float(gc))
        nc.vector.scalar_tensor_tensor(
            out1, i1, float(gd), out1, mb.AluOpType.mult, mb.AluOpType.add
        )


def _get_nc(S: int, bfly):
    key = (S, bfly)
    if key not in _NC_CACHE:
        _NC_CACHE[key] = _build_nc(S, bfly)
    return _NC_CACHE[key]


def _build_L(Mr: np.ndarray, fold_scale: float) -> np.ndarray:
    """128x128 real matrix applying M on partition bits q0, q5, q10.

    Partition index p = q0*64 + q4*32 + q5*16 + q6*8 + q7*4 + q8*2 + q10.
    """
    I2 = np.eye(2, dtype=np.float64)
    L = np.array([[1.0]])
    for F in (Mr, I2, Mr, I2, I2, I2, Mr):  # q0, q4, q5, q6, q7, q8, q10
        L = np.kron(L, F)
    return L * fold_scale


def kernel(x: np.ndarray, M: np.ndarray) -> np.ndarray:
    x = np.asarray(x)
    M = np.asarray(M)
    n, batch = x.shape
    assert n == 1 << 24 and batch == 1, (n, batch)

    is_complex = np.iscomplexobj(x)
    if is_complex:
        xc = np.ascontiguousarray(x, dtype=np.complex64)
        xf = xc.reshape(-1).view(np.float32)
    else:
        xf = np.ascontiguousarray(x, dtype=np.float32).reshape(-1)
    xh = xf.astype(np.float16)  # wire format: fp16 (tolerance is 2e-2)
    F = xh.size
    S = int(round(math.log2(F))) - 3  # per-core slab = F/8 elems

    # gate matrix: must be (essentially) real
    Mc = np.asarray(M, dtype=np.complex128)
    assert np.abs(Mc.imag).max() <= 1e-5 * max(np.abs(Mc.real).max(), 1e-30), (
        "complex-valued M is not supported"
    )
    Mr = Mc.real.copy()

    s0 = Mr[0, 0]
    had_form = (
        abs(s0) > 0
        and abs(Mr[0, 1] - s0) <= 1e-6 * abs(s0)
        and abs(Mr[1, 0] - s0) <= 1e-6 * abs(s0)
        and abs(Mr[1, 1] + s0) <= 1e-6 * abs(s0)
    )
    if had_form:
        bfly = ("had",)
        L = _build_L(Mr, fold_scale=s0 * s0)  # two unnormalized butterflies
    else:
        bfly = ("gen", Mr[0, 0], Mr[0, 1], Mr[1, 0], Mr[1, 1])
        L = _build_L(Mr, fold_scale=1.0)
    wT = np.ascontiguousarray(L.T.astype(np.float16))  # lhsT[k, i] = L[i, k]

    nc = _get_nc(S, bfly if bfly[0] == "had" else bfly)

    # shard by qubits (1,2,3): xh.reshape(2[q0], 8[q1q2q3], F/16)
    xs = xh.reshape(2, 8, F // 16)
    in_maps = [
        {"x": np.ascontiguousarray(xs[:, cid, :]).reshape(-1), "w": wT}
        for cid in range(N_CORES)
    ]
    res = run_bass_kernel_spmd(nc, in_maps, list(range(N_CORES)))
    global LAST_RESULT
    LAST_RESULT = res
    outs = res.results

    yf = np.empty(F, dtype=np.float32)
    ys = yf.reshape(2, 8, F // 16)
    for cid in range(N_CORES):
        ys[:, cid, :] = outs[cid]["y"].reshape(2, F // 16).astype(np.float32)

    if is_complex:
        return yf.view(np.complex64).reshape(n, batch)
    return yf.reshape(n, batch)
